# revision 8
# baseline (speedup 1.0000x reference)
"""DeepSeek-MoE layer (top-2, capacity-dropped, shared gate) on 8 trn2 NeuronCores.

Expert-parallel: core c owns expert c's down-projection. Tokens are
dispatched by routing in capacity-slot space:
  - router logits (f32, exact) computed data-parallel, AllGathered (tiny),
    routing math replicated on every core
  - gate proj + SiLU computed data-parallel in bf16, hidden AllGathered
  - each core transpose-gathers its expert's <=640 assigned token rows of
    hidden, runs the down matmul, AllGathers z
  - each home core gathers its tokens' (<=2) slot rows of z and combines
    with the renormalized gate weights.
"""

import os
import sys

for _p in ("/opt/trn_rl_repo",):
    if _p not in sys.path:
        sys.path.append(_p)

import numpy as np

import concourse.bass as bass
import concourse.mybir as mybir
import concourse.tile as tile
from concourse import bacc
from concourse.bass_utils import run_bass_kernel_spmd
from concourse.masks import make_upper_triangular

F32 = mybir.dt.float32
BF16 = mybir.dt.bfloat16
I16 = mybir.dt.int16
AX = mybir.AxisListType
OP = mybir.AluOpType
ACTF = mybir.ActivationFunctionType
BF16_NP = mybir.dt.np(BF16)

D = 1024          # d_model
H = 2048          # d_hidden
E = 8             # experts = cores
TOPK = 2
N = 4096          # tokens (B*T)
NC = 8            # cores
TPC = N // NC     # tokens per core = 512
CAP = 640         # ceil(N / E * 1.25)
NT = N // 128     # 32 token tiles
TLOC = TPC // 128  # 4 token tiles per core
NSLOT = CAP       # slots per expert
ZROWS = E * CAP   # 5120

_CACHED = None


def _build():
    nc = bacc.Bacc(None, target_bir_lowering=False, debug=False)

    # ---- I/O ----
    xT = nc.dram_tensor("xT", [D, TPC], F32, kind="ExternalInput")
    w_rT = nc.dram_tensor("w_rT", [D, E], F32, kind="ExternalInput")
    w_gT = nc.dram_tensor("w_gT", [D, H], BF16, kind="ExternalInput")
    w_dT = nc.dram_tensor("w_dT", [H, D], BF16, kind="ExternalInput")
    myhot = nc.dram_tensor("myhot", [128, E], F32, kind="ExternalInput")
    tsel = nc.dram_tensor("tsel", [128, NT], F32, kind="ExternalInput")
    y = nc.dram_tensor("y", [TPC, D], F32, kind="ExternalOutput")
    dbg = None
    if os.environ.get("MOE_DBG"):
        dbg = nc.dram_tensor("dbg", [128, NT, E], F32, kind="ExternalOutput")
        dbgB = nc.dram_tensor("dbgB", [16, 128], F32, kind="ExternalOutput")
        dbgC = nc.dram_tensor("dbgC", [128, NT, 2], F32, kind="ExternalOutput")
        dbgD = nc.dram_tensor("dbgD", [128, TLOC, 2], F32, kind="ExternalOutput")
        dbgZ = nc.dram_tensor("dbgZ", [128, NSLOT // 128, D], BF16, kind="ExternalOutput")
        dbgT = nc.dram_tensor("dbgT", [768, 64], F32, kind="ExternalOutput")
        dbgQ = nc.dram_tensor("dbgQ", [128, NT, 4], F32, kind="ExternalOutput")
        dbgG = nc.dram_tensor("dbgG", [128, 2 * TLOC, D], BF16, kind="ExternalOutput")

    # ---- internal DRAM (collective bounce + tables) ----
    lg_in = nc.dram_tensor("lg_in", [128 * TLOC * E], F32)
    lg_out = nc.dram_tensor("lg_out", [NC * 128 * TLOC * E], F32, addr_space="Shared")
    hid_in = nc.dram_tensor("hid_in", [TPC, H], BF16)
    hid_out = nc.dram_tensor("hid_out", [N, H], BF16, addr_space="Shared")
    z_in = nc.dram_tensor("z_in", [NSLOT, D], BF16)
    z_out = nc.dram_tensor("z_out", [ZROWS, D], BF16, addr_space="Shared")
    tabx = nc.dram_tensor("tabx", [768, 64], F32)

    rg = [list(range(NC))]

    with tile.TileContext(nc) as tc:
        with (
            tc.tile_pool(name="const", bufs=1) as cpool,
            tc.tile_pool(name="wts", bufs=1) as wpool,
            tc.tile_pool(name="rt", bufs=1) as rpool,
            tc.tile_pool(name="work", bufs=1) as wk,
            tc.tile_pool(name="ps_a", bufs=2, space="PSUM") as psa,
            tc.tile_pool(name="ps_b", bufs=2, space="PSUM") as psb,
        ):
            # ================= loads & constants =================
            xT_sb = wpool.tile([128, D // 128, TPC], F32)
            nc.sync.dma_start(xT_sb[:], xT.rearrange("(k p) n -> p k n", p=128))
            wr_sb = wpool.tile([128, D // 128, E], F32)
            nc.sync.dma_start(wr_sb[:], w_rT.rearrange("(k p) e -> p k e", p=128))
            wg_sb = wpool.tile([128, D // 128, H], BF16)
            nc.sync.dma_start(wg_sb[:], w_gT.rearrange("(k p) h -> p k h", p=128))
            wd_sb = wpool.tile([128, H // 128, D], BF16)
            nc.sync.dma_start(wd_sb[:], w_dT.rearrange("(k p) d -> p k d", p=128))
            myhot_sb = cpool.tile([128, E], F32)
            nc.sync.dma_start(myhot_sb[:], myhot[:])
            tsel_sb = cpool.tile([128, NT], F32)
            nc.sync.dma_start(tsel_sb[:], tsel[:])

            ut = cpool.tile([128, 128], F32)        # ut[k, m] = 1 if m >= k
            make_upper_triangular(nc, ut[:], val=1.0, diag=True)
            onesm = cpool.tile([128, 128], F32)
            nc.gpsimd.memset(onesm[:], 1.0)
            ident = cpool.tile([128, 128], F32)      # identity for perm matmuls
            nc.gpsimd.memset(ident[:], 1.0)
            nc.gpsimd.affine_select(
                out=ident[:], in_=ident[:], compare_op=OP.is_equal, fill=0.0,
                base=0, pattern=[[-1, 128]], channel_multiplier=1)
            eidx = cpool.tile([128, E], F32)         # column e -> e
            nc.gpsimd.iota(eidx[:], pattern=[[1, E]], base=0, channel_multiplier=0,
                           allow_small_or_imprecise_dtypes=True)
            iota64 = cpool.tile([128, 64], F32)      # column f -> f
            nc.gpsimd.iota(iota64[:], pattern=[[1, 64]], base=0, channel_multiplier=0,
                           allow_small_or_imprecise_dtypes=True)
            thr40 = cpool.tile([128, 40], F32)       # 16*(s+1), s = 0..39
            nc.gpsimd.iota(thr40[:], pattern=[[16, 40]], base=16, channel_multiplier=0,
                           allow_small_or_imprecise_dtypes=True)
            trash = cpool.tile([128, 1], F32)        # 640 + p
            nc.gpsimd.iota(trash[:], pattern=[[0, 1]], base=640, channel_multiplier=1,
                           allow_small_or_imprecise_dtypes=True)
            tokid = cpool.tile([128, NT], F32)       # token id = t*128 + p
            nc.gpsimd.iota(tokid[:], pattern=[[128, NT]], base=0, channel_multiplier=1,
                           allow_small_or_imprecise_dtypes=True)
            zeros32 = cpool.tile([128, NT], F32)
            nc.vector.memset(zeros32[:], 0.0)

            # ================= gate proj + SiLU (data-parallel) =================
            xTb = wpool.tile([128, D // 128, TPC], BF16)
            nc.vector.tensor_copy(xTb[:], xT_sb[:])
            hid_sb = rpool.tile([128, TLOC, H], BF16)
            for mt in range(TLOC):
                for nch in range(H // 512):
                    ps = psa.tile([128, 512], F32, tag="ps512")
                    for k in range(D // 128):
                        nc.tensor.matmul(
                            ps[:], xTb[:, k, mt * 128:(mt + 1) * 128],
                            wg_sb[:, k, nch * 512:(nch + 1) * 512],
                            start=(k == 0), stop=(k == D // 128 - 1))
                    nc.scalar.activation(
                        hid_sb[:, mt, nch * 512:(nch + 1) * 512], ps[:], ACTF.Silu)
            nc.sync.dma_start(
                hid_in.rearrange("(t p) h -> p t h", p=128), hid_sb[:])
            nc.gpsimd.collective_compute(
                "AllGather", OP.bypass, replica_groups=rg,
                ins=[hid_in[:].opt()], outs=[hid_out[:].opt()])

            # ================= router (f32) =================
            lg_sb = wk.tile([128, TLOC, E], F32)
            for mt in range(TLOC):
                ps = psa.tile([128, 512], F32, tag="ps512")
                for k in range(D // 128):
                    nc.tensor.matmul(
                        ps[:, :E], xT_sb[:, k, mt * 128:(mt + 1) * 128],
                        wr_sb[:, k, :], start=(k == 0), stop=(k == D // 128 - 1))
                nc.vector.tensor_copy(lg_sb[:, mt, :], ps[:, :E])
            nc.sync.dma_start(lg_in.rearrange("(p t e) -> p (t e)", p=128, t=TLOC, e=E),
                              lg_sb[:])
            nc.gpsimd.collective_compute(
                "AllGather", OP.bypass, replica_groups=rg,
                ins=[lg_in[:].opt()], outs=[lg_out[:].opt()])
            lg = rpool.tile([128, NT, E], F32)
            nc.sync.dma_start(
                lg[:].rearrange("p (c t) e -> p c t e", c=NC),
                lg_out.rearrange("(c p t e) -> p c t e", p=128, t=TLOC, e=E))

            # ================= routing math (replicated) =================
            def b3(ap_pt1, last=E):  # [128, NT, 1] -> [128, NT, last]
                return ap_pt1.broadcast_to([128, NT, last])

            rmax = wk.tile([128, NT, 1], F32, tag="r1")
            nc.vector.reduce_max(rmax[:], lg[:], axis=AX.X)
            xs = wk.tile([128, NT, E], F32, tag="rE")
            nc.vector.tensor_sub(xs[:], lg[:], b3(rmax[:]))
            ex = wk.tile([128, NT, E], F32, tag="rE2")
            nc.scalar.activation(ex[:], xs[:], ACTF.Exp)
            esum = wk.tile([128, NT, 1], F32, tag="r2")
            nc.vector.reduce_sum(esum[:], ex[:], axis=AX.X)
            einv = wk.tile([128, NT, 1], F32, tag="r3")
            nc.vector.reciprocal(einv[:], esum[:])
            gates = rpool.tile([128, NT, E], F32)
            nc.vector.tensor_mul(gates[:], ex[:], b3(einv[:]))

            g0 = rpool.tile([128, NT, 1], F32)
            nc.vector.reduce_max(g0[:], gates[:], axis=AX.X)
            eq1 = rpool.tile([128, NT, E], F32)
            nc.vector.tensor_tensor(out=eq1[:], in0=gates[:], in1=b3(g0[:]),
                                    op=OP.is_equal)
            gm = wk.tile([128, NT, E], F32, tag="rE")
            nc.vector.tensor_scalar(out=gm[:], in0=eq1[:], scalar1=-2.0, scalar2=None,
                                    op0=OP.mult)
            nc.vector.tensor_add(gm[:], gm[:], gates[:])
            g1 = rpool.tile([128, NT, 1], F32)
            nc.vector.reduce_max(g1[:], gm[:], axis=AX.X)
            eq2 = rpool.tile([128, NT, E], F32)
            nc.vector.tensor_tensor(out=eq2[:], in0=gm[:], in1=b3(g1[:]),
                                    op=OP.is_equal)
            mask = rpool.tile([128, NT, E], F32)
            nc.vector.tensor_add(mask[:], eq1[:], eq2[:])

            # cumulative position: per-tile (partition) cumsum via triangular
            # matmul + cross-tile prefix via free-dim scans
            pm = rpool.tile([128, NT, E], F32)
            nc.vector.memset(pm[:, 0, :], 0.0)
            for e in range(E):
                nc.vector.tensor_tensor_scan(
                    pm[:, 1:NT, e], mask[:, 0:NT - 1, e], zeros32[:, 0:NT - 1],
                    0.0, OP.add, OP.add)
            ps_pos = psa.tile([128, 512], F32, tag="ps512")
            nc.tensor.matmul(ps_pos[:, :NT * E], ut[:],
                             mask[:].rearrange("p t e -> p (t e)"),
                             start=True, stop=False)
            nc.tensor.matmul(ps_pos[:, :NT * E], onesm[:],
                             pm[:].rearrange("p t e -> p (t e)"),
                             start=False, stop=True)
            posi = rpool.tile([128, NT, E], F32)   # inclusive cumsum
            nc.vector.tensor_copy(posi[:], ps_pos[:, :NT * E].rearrange(
                "p (t e) -> p t e", t=NT))

            kcap = rpool.tile([128, NT, E], F32)   # survives capacity & masked
            nc.vector.tensor_scalar(out=kcap[:], in0=posi[:], scalar1=float(CAP),
                                    scalar2=None, op0=OP.is_le)
            nc.vector.tensor_mul(kcap[:], kcap[:], mask[:])
            slot = rpool.tile([128, NT, E], F32)   # slot index when kept
            nc.vector.tensor_scalar(out=slot[:], in0=posi[:], scalar1=-1.0,
                                    scalar2=None, op0=OP.add)

            tmpE = wk.tile([128, NT, E], F32, tag="rE")
            k0 = rpool.tile([128, NT, 1], F32)
            nc.vector.tensor_mul(tmpE[:], eq1[:], kcap[:])
            nc.vector.reduce_sum(k0[:], tmpE[:], axis=AX.X)
            k1 = rpool.tile([128, NT, 1], F32)
            nc.vector.tensor_mul(tmpE[:], eq2[:], kcap[:])
            nc.vector.reduce_sum(k1[:], tmpE[:], axis=AX.X)

            gk0 = wk.tile([128, NT, 1], F32, tag="r4")
            nc.vector.tensor_mul(gk0[:], g0[:], k0[:])
            gk1 = wk.tile([128, NT, 1], F32, tag="r5")
            nc.vector.tensor_mul(gk1[:], g1[:], k1[:])
            den = wk.tile([128, NT, 1], F32, tag="r6")
            nc.vector.tensor_add(den[:], gk0[:], gk1[:])
            nc.vector.tensor_scalar(out=den[:], in0=den[:], scalar1=1e-6,
                                    scalar2=None, op0=OP.add)
            dinv = wk.tile([128, NT, 1], F32, tag="r7")
            nc.vector.reciprocal(dinv[:], den[:])
            w0 = rpool.tile([128, NT, 1], F32)
            nc.vector.tensor_mul(w0[:], gk0[:], dinv[:])
            w1 = rpool.tile([128, NT, 1], F32)
            nc.vector.tensor_mul(w1[:], gk1[:], dinv[:])

            eidx_b = eidx[:][:, None, :].broadcast_to([128, NT, E])
            e0 = wk.tile([128, NT, 1], F32, tag="r8")
            nc.vector.tensor_mul(tmpE[:], eidx_b, eq1[:])
            nc.vector.reduce_sum(e0[:], tmpE[:], axis=AX.X)
            e1 = wk.tile([128, NT, 1], F32, tag="r9")
            nc.vector.tensor_mul(tmpE[:], eidx_b, eq2[:])
            nc.vector.reduce_sum(e1[:], tmpE[:], axis=AX.X)
            s0 = wk.tile([128, NT, 1], F32, tag="r10")
            nc.vector.tensor_mul(tmpE[:], slot[:], eq1[:])
            nc.vector.tensor_mul(tmpE[:], tmpE[:], kcap[:])
            nc.vector.reduce_sum(s0[:], tmpE[:], axis=AX.X)
            s1 = wk.tile([128, NT, 1], F32, tag="r11")
            nc.vector.tensor_mul(tmpE[:], slot[:], eq2[:])
            nc.vector.tensor_mul(tmpE[:], tmpE[:], kcap[:])
            nc.vector.reduce_sum(s1[:], tmpE[:], axis=AX.X)
            # flat slot ids into z_out; 0 when dropped (w=0 kills it)
            flat0 = rpool.tile([128, NT], F32)
            nc.vector.tensor_scalar(out=flat0[:], in0=e0[:][:, :, 0],
                                    scalar1=float(CAP), scalar2=None, op0=OP.mult)
            nc.vector.tensor_add(flat0[:], flat0[:], s0[:][:, :, 0])
            nc.vector.tensor_mul(flat0[:], flat0[:], k0[:][:, :, 0])
            flat1 = rpool.tile([128, NT], F32)
            nc.vector.tensor_scalar(out=flat1[:], in0=e1[:][:, :, 0],
                                    scalar1=float(CAP), scalar2=None, op0=OP.mult)
            nc.vector.tensor_add(flat1[:], flat1[:], s1[:][:, :, 0])
            nc.vector.tensor_mul(flat1[:], flat1[:], k1[:][:, :, 0])

            if dbg is not None:
                dbg_sb = rpool.tile([128, NT, E], F32)
                nc.vector.tensor_mul(dbg_sb[:], kcap[:], posi[:])
                nc.sync.dma_start(dbg[:], dbg_sb[:])

            # ============ my-expert slot scatter (slot_token table) ============
            selm = wk.tile([128, NT, E], F32, tag="rE")  # kcap & my expert
            myhot_b = myhot_sb[:][:, None, :].broadcast_to([128, NT, E])
            nc.vector.tensor_mul(selm[:], kcap[:], myhot_b)
            selflag = wk.tile([128, NT], F32, tag="sf")
            nc.vector.reduce_sum(selflag[:], selm[:], axis=AX.X)
            posm = wk.tile([128, NT], F32, tag="pm")
            nc.vector.tensor_mul(selm[:], selm[:], slot[:])
            nc.vector.reduce_sum(posm[:], selm[:], axis=AX.X)
            # dst row: slot if selected else trash (640 + p)
            xdst = wk.tile([128, NT], F32, tag="xd")
            trash_b = trash[:].broadcast_to([128, NT])
            nc.vector.tensor_sub(xdst[:], posm[:], trash_b)
            nc.vector.tensor_mul(xdst[:], xdst[:], selflag[:])
            nc.vector.tensor_add(xdst[:], xdst[:], trash_b)
            # value rows: tokid in col 0
            xoh = wk.tile([128, NT, 64], F32, tag="xoh")
            nc.vector.memset(xoh[:], 0.0)
            nc.vector.tensor_copy(xoh[:, :, 0], tokid[:])

            # zero table, wrap scatter idx via perm matmuls, scatter, read back
            z64 = wk.tile([128, 64], F32, tag="z64")
            nc.vector.memset(z64[:], 0.0)
            for r in range(6):
                nc.sync.dma_start(tabx[r * 128:(r + 1) * 128, :], z64[:])
            xdst_w = wk.tile([16, NT * E], F32, tag="xw")   # wrapped [16, 256]
            for g in range(8):
                psp = psa.tile([16, 512], F32, tag="psw")
                nc.tensor.matmul(psp[:, :NT], ident[:, g * 16:(g + 1) * 16],
                                 xdst[:], start=True, stop=True)
                nc.vector.tensor_copy(
                    xdst_w[:].rearrange("q (t g) -> q t g", g=8)[:, :, g], psp[:, :NT])
            xdst_i = wk.tile([16, NT * E], I16, tag="xwi")
            nc.vector.tensor_copy(xdst_i[:], xdst_w[:])
            sidx = wk.tile([128, NT * E], I16, tag="sidx")
            for g in range(8):
                nc.sync.dma_start(sidx[16 * g:16 * (g + 1), :], xdst_i[:])
            nc.gpsimd.dma_scatter_add(tabx[:], xoh[:], sidx[:], N, N, 64)
            if dbg is not None:
                nc.sync.dma_start(dbgT[:], tabx[:])
                dbgQ_sb = wk.tile([128, NT, 4], F32, tag="dbgQ")
                nc.vector.tensor_copy(dbgQ_sb[:, :, 0], posm[:])
                nc.vector.tensor_copy(dbgQ_sb[:, :, 1], selflag[:])
                nc.vector.tensor_copy(dbgQ_sb[:, :, 2], xdst[:])
                nc.vector.tensor_copy(dbgQ_sb[:, :, 3], xdst[:])
                nc.sync.dma_start(dbgQ[:], dbgQ_sb[:])
            # read back slot rows col0, wrapped -> [16, 40]
            xidx_f = wk.tile([16, 40], F32, tag="xif")
            nc.sync.dma_start(
                xidx_f[:], tabx.rearrange("(s q) e -> q s e", q=16)[:, 0:40, 0])

            # ============ y-phase idx arrays (flat0/flat1 of my tokens) ============
            fm = wk.tile([128, NT], F32, tag="fm")
            idxy_f = wk.tile([16, 64], F32, tag="yif")
            for kk, flat in ((0, flat0), (1, flat1)):
                nc.vector.tensor_mul(fm[:], flat[:], tsel_sb[:])
                fmy = wk.tile([128, TLOC], F32, tag="fmy")
                nc.vector.reduce_sum(
                    fmy[:], fm[:].rearrange("p (g i) -> p i g", g=NC), axis=AX.X)
                for g in range(8):
                    psp = psa.tile([16, 512], F32, tag="psw")
                    nc.tensor.matmul(psp[:, :TLOC], ident[:, g * 16:(g + 1) * 16],
                                     fmy[:], start=True, stop=True)
                    nc.vector.tensor_copy(
                        idxy_f[:].rearrange("q (k t g) -> q k t g", k=2, g=8)[:, kk, :, g],
                        psp[:, :TLOC])
            # combine idx arrays -> int16, replicate to 128 partitions
            idx_i = wk.tile([16, 128], I16, tag="idxi")
            nc.vector.tensor_copy(idx_i[:, 0:40], xidx_f[:])
            nc.vector.tensor_copy(idx_i[:, 64:128], idxy_f[:])
            idx_rep = rpool.tile([128, 128], I16)
            for g in range(8):
                nc.sync.dma_start(idx_rep[16 * g:16 * (g + 1), :], idx_i[:])
            if dbg is not None:
                dbgB_sb = wk.tile([16, 128], F32, tag="dbgB")
                nc.vector.tensor_copy(dbgB_sb[:], idx_i[:])
                nc.sync.dma_start(dbgB[:], dbgB_sb[:])
                dbgC_sb = wk.tile([128, NT, 2], F32, tag="dbgC")
                nc.vector.tensor_copy(dbgC_sb[:, :, 0], flat0[:])
                nc.vector.tensor_copy(dbgC_sb[:, :, 1], flat1[:])
                nc.sync.dma_start(dbgC[:], dbgC_sb[:])

            # ================= expert gather + down matmul =================
            hsel = rpool.tile([128, H // 128, NSLOT], BF16, tag="late1")
            nc.gpsimd.dma_gather(hsel[:], hid_out[:], idx_rep[:, 0:40],
                                 NSLOT, NSLOT, H, transpose=True)
            z_sb = rpool.tile([128, NSLOT // 128, D], BF16, tag="late2")
            for m in range(NSLOT // 128):
                psd = psb.tile([128, D], F32, tag="psd")
                for nch in range(D // 512):
                    for k in range(H // 128):
                        nc.tensor.matmul(
                            psd[:, nch * 512:(nch + 1) * 512],
                            hsel[:, k, m * 128:(m + 1) * 128],
                            wd_sb[:, k, nch * 512:(nch + 1) * 512],
                            start=(k == 0), stop=(k == H // 128 - 1))
                nc.scalar.copy(z_sb[:, m, :], psd[:])
            if dbg is not None:
                nc.sync.dma_start(dbgZ[:], z_sb[:])
            nc.sync.dma_start(z_in.rearrange("(c p) d -> p c d", p=128), z_sb[:])
            nc.gpsimd.collective_compute(
                "AllGather", OP.bypass, replica_groups=rg,
                ins=[z_in[:].opt()], outs=[z_out[:].opt()])

            # ================= home-core combine =================
            zg = rpool.tile([128, 2 * TLOC, D], BF16, tag="late1")
            nc.gpsimd.dma_gather(zg[:], z_out[:], idx_rep[:, 64:128],
                                 2 * TPC, 2 * TPC, D, transpose=False)
            w0my = wk.tile([128, TLOC], F32, tag="w0m")
            nc.vector.tensor_mul(fm[:], w0[:][:, :, 0], tsel_sb[:])
            nc.vector.reduce_sum(
                w0my[:], fm[:].rearrange("p (g i) -> p i g", g=NC), axis=AX.X)
            w1my = wk.tile([128, TLOC], F32, tag="w1m")
            nc.vector.tensor_mul(fm[:], w1[:][:, :, 0], tsel_sb[:])
            nc.vector.reduce_sum(
                w1my[:], fm[:].rearrange("p (g i) -> p i g", g=NC), axis=AX.X)
            if dbg is not None:
                nc.sync.dma_start(dbgG[:], zg[:])
                dbgD_sb = wk.tile([128, TLOC, 2], F32, tag="dbgD")
                nc.vector.tensor_copy(dbgD_sb[:, :, 0], w0my[:])
                nc.vector.tensor_copy(dbgD_sb[:, :, 1], w1my[:])
                nc.sync.dma_start(dbgD[:], dbgD_sb[:])
            y_sb = rpool.tile([128, TLOC, D], F32, tag="late2")
            yt = wk.tile([128, D], F32, tag="yt")
            for t in range(TLOC):
                nc.vector.tensor_scalar(out=y_sb[:, t, :], in0=zg[:, t, :],
                                        scalar1=w0my[:, t:t + 1], scalar2=None,
                                        op0=OP.mult)
                nc.vector.tensor_scalar(out=yt[:], in0=zg[:, TLOC + t, :],
                                        scalar1=w1my[:, t:t + 1], scalar2=None,
                                        op0=OP.mult)
                nc.vector.tensor_add(y_sb[:, t, :], y_sb[:, t, :], yt[:])
            nc.sync.dma_start(y.rearrange("(t p) d -> p t d", p=128), y_sb[:])

    nc.compile()
    return nc


def _get_nc():
    global _CACHED
    if _CACHED is None:
        _CACHED = _build()
    return _CACHED


def kernel(x, w_router, w_gate, w_down):
    x = np.asarray(x)
    w_router = np.asarray(w_router)
    w_gate = np.asarray(w_gate)
    w_down = np.asarray(w_down)
    B, T, _ = x.shape
    xf = np.ascontiguousarray(x.reshape(N, D).astype(np.float32))
    w_rT = np.ascontiguousarray(w_router.astype(np.float32).T)
    w_gT = np.ascontiguousarray(w_gate.astype(np.float32).T.astype(BF16_NP))

    nc = _get_nc()
    in_maps = []
    for c in range(NC):
        xT_c = np.ascontiguousarray(xf[c * TPC:(c + 1) * TPC].T)
        w_dT_c = np.ascontiguousarray(
            w_down[c].astype(np.float32).T.astype(BF16_NP))
        myhot = np.zeros((128, E), dtype=np.float32)
        myhot[:, c] = 1.0
        tsel = np.zeros((128, NT), dtype=np.float32)
        tsel[:, c * TLOC:(c + 1) * TLOC] = 1.0
        in_maps.append({
            "xT": xT_c, "w_rT": w_rT, "w_gT": w_gT, "w_dT": w_dT_c,
            "myhot": myhot, "tsel": tsel,
        })
    res = run_bass_kernel_spmd(nc, in_maps, core_ids=list(range(NC)),
                               trace=bool(os.environ.get("MOE_TRACE")))
    kernel.last_results = res
    y = np.concatenate([res.results[c]["y"] for c in range(NC)], axis=0)
    return y.reshape(B, T, D).astype(x.dtype)


# revision 10
# speedup vs baseline: 1.1669x; 1.1669x over previous
"""DeepSeek-MoE layer (top-2, capacity-dropped, shared gate) on 8 trn2 NeuronCores.

Expert-parallel, x-dispatch: core c owns expert c's down-projection.
  - router logits (f32, exact) computed data-parallel, AllGathered (tiny),
    routing math replicated on every core
  - slot->token map built by a collision-free dma_scatter_add table
  - each core transpose-gathers its expert's <=640 assigned token rows of
    x (bf16), runs gate+SiLU and the down matmul for those slots,
    AllGathers z (2 chunks, overlapping the down matmul)
  - each home core gathers its tokens' (<=2) slot rows of z and combines
    with the renormalized gate weights.
"""

import os
import sys

for _p in ("/opt/trn_rl_repo",):
    if _p not in sys.path:
        sys.path.append(_p)

import numpy as np

import concourse.bass as bass
import concourse.mybir as mybir
import concourse.tile as tile
from concourse import bacc
from concourse.bass_utils import run_bass_kernel_spmd
from concourse.masks import make_upper_triangular

F32 = mybir.dt.float32
BF16 = mybir.dt.bfloat16
I16 = mybir.dt.int16
AX = mybir.AxisListType
OP = mybir.AluOpType
ACTF = mybir.ActivationFunctionType
BF16_NP = mybir.dt.np(BF16)

D = 1024          # d_model
H = 2048          # d_hidden
E = 8             # experts = cores
N = 4096          # tokens (B*T)
NC = 8            # cores
TPC = N // NC     # tokens per core = 512
CAP = 640         # ceil(N / E * 1.25)
NT = N // 128     # 32 token tiles
TLOC = TPC // 128  # 4 token tiles per core
CAPA = 256        # z chunk A slots (m-tiles 0..1)
CAPB = CAP - CAPA  # chunk B slots (m-tiles 2..4)

_CACHED = None


def _build():
    nc = bacc.Bacc(None, target_bir_lowering=False, debug=False)

    # ---- I/O ----
    xT = nc.dram_tensor("xT", [D, TPC], F32, kind="ExternalInput")
    x_bf = nc.dram_tensor("x_bf", [N, D], BF16, kind="ExternalInput")
    w_rT = nc.dram_tensor("w_rT", [D, E], F32, kind="ExternalInput")
    w_gT = nc.dram_tensor("w_gT", [D, H], BF16, kind="ExternalInput")
    w_dT = nc.dram_tensor("w_dT", [H, D], BF16, kind="ExternalInput")
    myhot = nc.dram_tensor("myhot", [128, E], F32, kind="ExternalInput")
    tsel = nc.dram_tensor("tsel", [128, NT], F32, kind="ExternalInput")
    y = nc.dram_tensor("y", [TPC, D], F32, kind="ExternalOutput")
    dbg = None
    if os.environ.get("MOE_DBG"):
        dbg = nc.dram_tensor("dbg", [128, NT, E], F32, kind="ExternalOutput")
        dbgB = nc.dram_tensor("dbgB", [16, 128], F32, kind="ExternalOutput")
        dbgZ = nc.dram_tensor("dbgZ", [128, CAP // 128, D], BF16,
                              kind="ExternalOutput")

    # ---- internal DRAM ----
    lg_in = nc.dram_tensor("lg_in", [128 * TLOC * E], F32)
    lg_out = nc.dram_tensor("lg_out", [NC * 128 * TLOC * E], F32, addr_space="Shared")
    z_inA = nc.dram_tensor("z_inA", [CAPA, D], BF16)
    z_inB = nc.dram_tensor("z_inB", [CAPB, D], BF16)
    z_out = nc.dram_tensor("z_out", [NC * CAP, D], BF16, addr_space="Shared")
    tabx = nc.dram_tensor("tabx", [768 + N, 64], F32)

    rg = [list(range(NC))]

    with tile.TileContext(nc) as tc:
        with (
            tc.tile_pool(name="const", bufs=1) as cpool,
            tc.tile_pool(name="wts", bufs=1) as wpool,
            tc.tile_pool(name="rt", bufs=1) as rpool,
            tc.tile_pool(name="work", bufs=1) as wk,
            tc.tile_pool(name="psum", bufs=2, space="PSUM") as psp,
        ):
            # ================= loads & constants =================
            xT_sb = wpool.tile([128, D // 128, TPC], F32)
            nc.sync.dma_start(xT_sb[:], xT.rearrange("(k p) n -> p k n", p=128))
            wr_sb = wpool.tile([128, D // 128, E], F32)
            nc.sync.dma_start(wr_sb[:], w_rT.rearrange("(k p) e -> p k e", p=128))
            wg_sb = wpool.tile([128, D // 128, H], BF16)
            nc.sync.dma_start(wg_sb[:], w_gT.rearrange("(k p) h -> p k h", p=128))
            wd_sb = wpool.tile([128, H // 128, D], BF16)
            nc.sync.dma_start(wd_sb[:], w_dT.rearrange("(k p) d -> p k d", p=128))
            myhot_sb = cpool.tile([128, E], F32)
            nc.sync.dma_start(myhot_sb[:], myhot[:])
            tsel_sb = cpool.tile([128, NT], F32)
            nc.sync.dma_start(tsel_sb[:], tsel[:])

            ut = cpool.tile([128, 128], F32)        # ut[k, m] = 1 if m >= k
            make_upper_triangular(nc, ut[:], val=1.0, diag=True)
            onesm = cpool.tile([128, 128], F32)
            nc.gpsimd.memset(onesm[:], 1.0)
            ident = cpool.tile([128, 128], F32)      # identity for perm matmuls
            nc.gpsimd.memset(ident[:], 1.0)
            nc.gpsimd.affine_select(
                out=ident[:], in_=ident[:], compare_op=OP.is_equal, fill=0.0,
                base=0, pattern=[[-1, 128]], channel_multiplier=1)
            eidx = cpool.tile([128, E], F32)         # column e -> e
            nc.gpsimd.iota(eidx[:], pattern=[[1, E]], base=0, channel_multiplier=0,
                           allow_small_or_imprecise_dtypes=True)
            tokid = cpool.tile([128, NT], F32)       # token id = t*128 + p
            nc.gpsimd.iota(tokid[:], pattern=[[128, NT]], base=0, channel_multiplier=1,
                           allow_small_or_imprecise_dtypes=True)
            zeros32 = cpool.tile([128, NT], F32)
            nc.vector.memset(zeros32[:], 0.0)
            # zero table rows 0..767 early (trash rows are never read)
            z64 = cpool.tile([128, 64], F32)
            nc.vector.memset(z64[:], 0.0)
            for r in range(6):
                nc.sync.dma_start(tabx[r * 128:(r + 1) * 128, :], z64[:])

            # ================= router (f32) -> AllGather =================
            lg_sb = wk.tile([128, TLOC, E], F32)
            for mt in range(TLOC):
                ps = psp.tile([128, 640], F32, tag="pa")
                for k in range(D // 128):
                    nc.tensor.matmul(
                        ps[:, :E], xT_sb[:, k, mt * 128:(mt + 1) * 128],
                        wr_sb[:, k, :], start=(k == 0), stop=(k == D // 128 - 1))
                nc.vector.tensor_copy(lg_sb[:, mt, :], ps[:, :E])
            nc.sync.dma_start(lg_in.rearrange("(p t e) -> p (t e)", p=128, t=TLOC, e=E),
                              lg_sb[:])
            nc.gpsimd.collective_compute(
                "AllGather", OP.bypass, replica_groups=rg,
                ins=[lg_in[:].opt()], outs=[lg_out[:].opt()])
            lg = rpool.tile([128, NT, E], F32)
            nc.sync.dma_start(
                lg[:].rearrange("p (c t) e -> p c t e", c=NC),
                lg_out.rearrange("(c p t e) -> p c t e", p=128, t=TLOC, e=E))

            # ================= routing math (replicated) =================
            def b3(ap_pt1, last=E):
                return ap_pt1.broadcast_to([128, NT, last])

            rmax = wk.tile([128, NT, 1], F32, tag="r1")
            nc.vector.reduce_max(rmax[:], lg[:], axis=AX.X)
            xs = wk.tile([128, NT, E], F32, tag="xs")
            nc.vector.tensor_sub(xs[:], lg[:], b3(rmax[:]))
            ex = wk.tile([128, NT, E], F32, tag="ex")
            nc.scalar.activation(ex[:], xs[:], ACTF.Exp)
            esum = wk.tile([128, NT, 1], F32, tag="r2")
            nc.vector.reduce_sum(esum[:], ex[:], axis=AX.X)
            einv = wk.tile([128, NT, 1], F32, tag="r3")
            nc.vector.reciprocal(einv[:], esum[:])
            gates = rpool.tile([128, NT, E], F32)
            nc.vector.tensor_mul(gates[:], ex[:], b3(einv[:]))

            g0 = rpool.tile([128, NT, 1], F32)
            nc.vector.reduce_max(g0[:], gates[:], axis=AX.X)
            eq1 = rpool.tile([128, NT, E], F32)
            nc.vector.tensor_tensor(out=eq1[:], in0=gates[:], in1=b3(g0[:]),
                                    op=OP.is_equal)
            gm = wk.tile([128, NT, E], F32, tag="gm")
            nc.vector.scalar_tensor_tensor(out=gm[:], in0=eq1[:], scalar=-2.0,
                                           in1=gates[:], op0=OP.mult, op1=OP.add)
            g1 = rpool.tile([128, NT, 1], F32)
            nc.vector.reduce_max(g1[:], gm[:], axis=AX.X)
            eq2 = rpool.tile([128, NT, E], F32)
            nc.vector.tensor_tensor(out=eq2[:], in0=gm[:], in1=b3(g1[:]),
                                    op=OP.is_equal)
            mask = rpool.tile([128, NT, E], F32)
            nc.vector.tensor_add(mask[:], eq1[:], eq2[:])

            # global inclusive cumsum over tokens per expert
            pm = rpool.tile([128, NT, E], F32)
            nc.vector.memset(pm[:, 0, :], 0.0)
            for e in range(E):
                nc.vector.tensor_tensor_scan(
                    pm[:, 1:NT, e], mask[:, 0:NT - 1, e], zeros32[:, 0:NT - 1],
                    0.0, OP.add, OP.add)
            ps_pos = psp.tile([128, 640], F32, tag="pa")
            nc.tensor.matmul(ps_pos[:, :NT * E], ut[:],
                             mask[:].rearrange("p t e -> p (t e)"),
                             start=True, stop=False)
            nc.tensor.matmul(ps_pos[:, :NT * E], onesm[:],
                             pm[:].rearrange("p t e -> p (t e)"),
                             start=False, stop=True)
            posi = rpool.tile([128, NT, E], F32)
            nc.vector.tensor_copy(posi[:], ps_pos[:, :NT * E].rearrange(
                "p (t e) -> p t e", t=NT))

            kcap = rpool.tile([128, NT, E], F32)
            nc.vector.scalar_tensor_tensor(out=kcap[:], in0=posi[:],
                                           scalar=float(CAP), in1=mask[:],
                                           op0=OP.is_le, op1=OP.mult)
            slot = rpool.tile([128, NT, E], F32)
            nc.vector.tensor_scalar(out=slot[:], in0=posi[:], scalar1=-1.0,
                                    scalar2=None, op0=OP.add)

            # -------- my-expert selection -> scatter (critical path) --------
            myhot_b = myhot_sb[:][:, None, :].broadcast_to([128, NT, E])
            selm = wk.tile([128, NT, E], F32, tag="selm")
            nc.vector.tensor_mul(selm[:], kcap[:], myhot_b)
            selflag = wk.tile([128, NT], F32, tag="sf")
            nc.vector.reduce_sum(selflag[:], selm[:], axis=AX.X)
            posm = wk.tile([128, NT], F32, tag="pmy")
            nc.vector.tensor_mul(selm[:], selm[:], slot[:])
            nc.vector.reduce_sum(posm[:], selm[:], axis=AX.X)
            # dst row: slot if selected else unique trash row 768 + token
            xdst = wk.tile([128, NT], F32, tag="xd")
            trash = wk.tile([128, NT], F32, tag="tr")
            nc.vector.tensor_scalar(out=trash[:], in0=tokid[:], scalar1=768.0,
                                    scalar2=None, op0=OP.add)
            nc.vector.tensor_sub(xdst[:], posm[:], trash[:])
            nc.vector.tensor_mul(xdst[:], xdst[:], selflag[:])
            nc.vector.tensor_add(xdst[:], xdst[:], trash[:])
            # value rows: tokid in col 0
            xoh = wk.tile([128, NT, 64], F32, tag="xoh")
            nc.vector.memset(xoh[:], 0.0)
            nc.vector.tensor_copy(xoh[:, :, 0], tokid[:])
            # wrap xdst to the 16-partition idx layout via perm matmuls
            xdst_w = wk.tile([16, NT * E], F32, tag="xw")
            for g in range(8):
                psw = psp.tile([16, 640], F32, tag="pa")
                nc.tensor.matmul(psw[:, :NT], ident[:, g * 16:(g + 1) * 16],
                                 xdst[:], start=True, stop=True)
                nc.vector.tensor_copy(
                    xdst_w[:].rearrange("q (t g) -> q t g", g=8)[:, :, g],
                    psw[:, :NT])
            xdst_i = wk.tile([16, NT * E], I16, tag="xwi")
            nc.vector.tensor_copy(xdst_i[:], xdst_w[:])
            sidx = wk.tile([128, NT * E], I16, tag="sidx")
            for g in range(8):
                nc.sync.dma_start(sidx[16 * g:16 * (g + 1), :], xdst_i[:])
            nc.gpsimd.dma_scatter_add(tabx[:], xoh[:], sidx[:], N, N, 64)
            # read back slot rows col0, wrapped -> [16, 40]
            xidx_f = wk.tile([16, 40], F32, tag="xif")
            nc.sync.dma_start(
                xidx_f[:],
                tabx[0:768, :].rearrange("(s q) e -> q s e", q=16)[:, 0:40, 0])
            idx_i = wk.tile([16, 40], I16, tag="idxi")
            nc.vector.tensor_copy(idx_i[:], xidx_f[:])

            # ================= x gather + gate + down =================
            idx_rep = rpool.tile([128, 40], I16)
            for g in range(8):
                nc.sync.dma_start(idx_rep[16 * g:16 * (g + 1), :], idx_i[:])
            xsel = rpool.tile([128, D // 128, CAP], BF16)
            nc.gpsimd.dma_gather(xsel[:], x_bf[:], idx_rep[:],
                                 CAP, CAP, D, transpose=True)
            hsel = rpool.tile([128, H // 128, CAP], BF16)
            for h in range(H // 128):
                pg = psp.tile([128, 640], F32, tag="pa")
                for k in range(D // 128):
                    for lo, hi in ((0, 512), (512, CAP)):
                        nc.tensor.matmul(
                            pg[:, lo:hi], wg_sb[:, k, h * 128:(h + 1) * 128],
                            xsel[:, k, lo:hi],
                            start=(k == 0), stop=(k == D // 128 - 1))
                nc.scalar.activation(hsel[:, h, :], pg[:], ACTF.Silu)
            z_sb = rpool.tile([128, CAP // 128, D], BF16)
            for m in range(CAP // 128):
                psd = psp.tile([128, D], F32, tag="pb")
                for k in range(H // 128):
                    for nch in range(D // 512):
                        nc.tensor.matmul(
                            psd[:, nch * 512:(nch + 1) * 512],
                            hsel[:, k, m * 128:(m + 1) * 128],
                            wd_sb[:, k, nch * 512:(nch + 1) * 512],
                            start=(k == 0), stop=(k == H // 128 - 1))
                nc.scalar.copy(z_sb[:, m, :], psd[:])
                if m == 1:
                    nc.sync.dma_start(
                        z_inA.rearrange("(c p) d -> p c d", p=128),
                        z_sb[:, 0:2, :])
                    nc.gpsimd.collective_compute(
                        "AllGather", OP.bypass, replica_groups=rg,
                        ins=[z_inA[:].opt()],
                        outs=[z_out[0:NC * CAPA, :].opt()])
            nc.sync.dma_start(
                z_inB.rearrange("(c p) d -> p c d", p=128), z_sb[:, 2:5, :])
            nc.gpsimd.collective_compute(
                "AllGather", OP.bypass, replica_groups=rg,
                ins=[z_inB[:].opt()],
                outs=[z_out[NC * CAPA:NC * CAP, :].opt()])

            # ======== off-critical-path: combine weights + y idx arrays ========
            tmpE = wk.tile([128, NT, E], F32, tag="tmpE")
            k0 = rpool.tile([128, NT, 1], F32)
            nc.vector.tensor_mul(tmpE[:], eq1[:], kcap[:])
            nc.vector.reduce_sum(k0[:], tmpE[:], axis=AX.X)
            k1 = rpool.tile([128, NT, 1], F32)
            nc.vector.tensor_mul(tmpE[:], eq2[:], kcap[:])
            nc.vector.reduce_sum(k1[:], tmpE[:], axis=AX.X)
            gk0 = wk.tile([128, NT, 1], F32, tag="r4")
            nc.vector.tensor_mul(gk0[:], g0[:], k0[:])
            gk1 = wk.tile([128, NT, 1], F32, tag="r5")
            nc.vector.tensor_mul(gk1[:], g1[:], k1[:])
            den = wk.tile([128, NT, 1], F32, tag="r6")
            nc.vector.scalar_tensor_tensor(out=den[:], in0=gk0[:], scalar=1e-6,
                                           in1=gk1[:], op0=OP.add, op1=OP.add)
            dinv = wk.tile([128, NT, 1], F32, tag="r7")
            nc.vector.reciprocal(dinv[:], den[:])
            w0 = rpool.tile([128, NT, 1], F32)
            nc.vector.tensor_mul(w0[:], gk0[:], dinv[:])
            w1 = rpool.tile([128, NT, 1], F32)
            nc.vector.tensor_mul(w1[:], gk1[:], dinv[:])

            eidx_b = eidx[:][:, None, :].broadcast_to([128, NT, E])
            e0 = wk.tile([128, NT, 1], F32, tag="r8")
            nc.vector.tensor_mul(tmpE[:], eidx_b, eq1[:])
            nc.vector.reduce_sum(e0[:], tmpE[:], axis=AX.X)
            e1 = wk.tile([128, NT, 1], F32, tag="r9")
            nc.vector.tensor_mul(tmpE[:], eidx_b, eq2[:])
            nc.vector.reduce_sum(e1[:], tmpE[:], axis=AX.X)
            s0 = wk.tile([128, NT, 1], F32, tag="r10")
            nc.vector.tensor_mul(tmpE[:], slot[:], eq1[:])
            nc.vector.tensor_mul(tmpE[:], tmpE[:], kcap[:])
            nc.vector.reduce_sum(s0[:], tmpE[:], axis=AX.X)
            s1 = wk.tile([128, NT, 1], F32, tag="r11")
            nc.vector.tensor_mul(tmpE[:], slot[:], eq2[:])
            nc.vector.tensor_mul(tmpE[:], tmpE[:], kcap[:])
            nc.vector.reduce_sum(s1[:], tmpE[:], axis=AX.X)

            # flat z_out row ids (chunked layout), 0 when dropped
            flat0 = rpool.tile([128, NT], F32)
            flat1 = rpool.tile([128, NT], F32)
            fb = wk.tile([128, NT], F32, tag="fb")
            isB = wk.tile([128, NT], F32, tag="isB")
            for flat, ee, ss, kk in ((flat0, e0, s0, k0), (flat1, e1, s1, k1)):
                # base = e*CAPA + s; if s >= CAPA: += (NC-1)*CAPA + e*(CAPB-CAPA)
                nc.vector.scalar_tensor_tensor(
                    out=flat[:], in0=ee[:][:, :, 0], scalar=float(CAPA),
                    in1=ss[:][:, :, 0], op0=OP.mult, op1=OP.add)
                nc.vector.tensor_scalar(
                    out=fb[:], in0=ee[:][:, :, 0], scalar1=float(CAPB - CAPA),
                    scalar2=float((NC - 1) * CAPA), op0=OP.mult, op1=OP.add)
                nc.vector.tensor_scalar(out=isB[:], in0=ss[:][:, :, 0],
                                        scalar1=float(CAPA), scalar2=None,
                                        op0=OP.is_ge)
                nc.vector.tensor_mul(fb[:], fb[:], isB[:])
                nc.vector.tensor_add(flat[:], flat[:], fb[:])
                nc.vector.tensor_mul(flat[:], flat[:], kk[:][:, :, 0])

            fm = wk.tile([128, NT], F32, tag="fm")
            idxy_f = wk.tile([16, 64], F32, tag="yif")
            for kk, flat in ((0, flat0), (1, flat1)):
                nc.vector.tensor_mul(fm[:], flat[:], tsel_sb[:])
                fmy = wk.tile([128, TLOC], F32, tag="fmy")
                nc.vector.reduce_sum(
                    fmy[:], fm[:].rearrange("p (g i) -> p i g", g=NC), axis=AX.X)
                for g in range(8):
                    psw = psp.tile([16, 640], F32, tag="pa")
                    nc.tensor.matmul(psw[:, :TLOC], ident[:, g * 16:(g + 1) * 16],
                                     fmy[:], start=True, stop=True)
                    nc.vector.tensor_copy(
                        idxy_f[:].rearrange("q (k t g) -> q k t g", k=2, g=8)[:, kk, :, g],
                        psw[:, :TLOC])
            idxy_i = wk.tile([16, 64], I16, tag="yii")
            nc.vector.tensor_copy(idxy_i[:], idxy_f[:])
            idxy_rep = rpool.tile([128, 64], I16)
            for g in range(8):
                nc.sync.dma_start(idxy_rep[16 * g:16 * (g + 1), :], idxy_i[:])
            w0my = wk.tile([128, TLOC], F32, tag="w0m")
            nc.vector.tensor_mul(fm[:], w0[:][:, :, 0], tsel_sb[:])
            nc.vector.reduce_sum(
                w0my[:], fm[:].rearrange("p (g i) -> p i g", g=NC), axis=AX.X)
            w1my = wk.tile([128, TLOC], F32, tag="w1m")
            nc.vector.tensor_mul(fm[:], w1[:][:, :, 0], tsel_sb[:])
            nc.vector.reduce_sum(
                w1my[:], fm[:].rearrange("p (g i) -> p i g", g=NC), axis=AX.X)

            if dbg is not None:
                dbg_sb = rpool.tile([128, NT, E], F32)
                nc.vector.tensor_mul(dbg_sb[:], kcap[:], posi[:])
                nc.sync.dma_start(dbg[:], dbg_sb[:])
                dbgB_sb = wk.tile([16, 128], F32, tag="dbgB")
                nc.vector.memset(dbgB_sb[:], 0.0)
                nc.vector.tensor_copy(dbgB_sb[:, 0:40], idx_i[:])
                nc.vector.tensor_copy(dbgB_sb[:, 64:128], idxy_i[:])
                nc.sync.dma_start(dbgB[:], dbgB_sb[:])
                nc.sync.dma_start(dbgZ[:], z_sb[:])

            # ================= home-core combine =================
            zg = rpool.tile([128, 2 * TLOC, D], BF16)
            nc.gpsimd.dma_gather(zg[:], z_out[:], idxy_rep[:],
                                 2 * TPC, 2 * TPC, D, transpose=False)
            y_sb = rpool.tile([128, TLOC, D], F32)
            for t in range(TLOC):
                yt = wk.tile([128, D], F32, tag="yt")
                nc.scalar.mul(yt[:], zg[:, TLOC + t, :], w1my[:, t:t + 1])
                nc.vector.scalar_tensor_tensor(
                    out=y_sb[:, t, :], in0=zg[:, t, :], scalar=w0my[:, t:t + 1],
                    in1=yt[:], op0=OP.mult, op1=OP.add)
            nc.sync.dma_start(y.rearrange("(t p) d -> p t d", p=128), y_sb[:])

    nc.compile()
    return nc


def _get_nc():
    global _CACHED
    if _CACHED is None:
        _CACHED = _build()
    return _CACHED


def kernel(x, w_router, w_gate, w_down):
    x = np.asarray(x)
    w_router = np.asarray(w_router)
    w_gate = np.asarray(w_gate)
    w_down = np.asarray(w_down)
    B, T, _ = x.shape
    xf = np.ascontiguousarray(x.reshape(N, D).astype(np.float32))
    x_bf = np.ascontiguousarray(xf.astype(BF16_NP))
    w_rT = np.ascontiguousarray(w_router.astype(np.float32).T)
    w_gT = np.ascontiguousarray(w_gate.astype(np.float32).T.astype(BF16_NP))

    nc = _get_nc()
    in_maps = []
    for c in range(NC):
        xT_c = np.ascontiguousarray(xf[c * TPC:(c + 1) * TPC].T)
        w_dT_c = np.ascontiguousarray(
            w_down[c].astype(np.float32).T.astype(BF16_NP))
        myhot = np.zeros((128, E), dtype=np.float32)
        myhot[:, c] = 1.0
        tsel = np.zeros((128, NT), dtype=np.float32)
        tsel[:, c * TLOC:(c + 1) * TLOC] = 1.0
        in_maps.append({
            "xT": xT_c, "x_bf": x_bf, "w_rT": w_rT, "w_gT": w_gT,
            "w_dT": w_dT_c, "myhot": myhot, "tsel": tsel,
        })
    res = run_bass_kernel_spmd(nc, in_maps, core_ids=list(range(NC)),
                               trace=bool(os.environ.get("MOE_TRACE")))
    kernel.last_results = res
    y = np.concatenate([res.results[c]["y"] for c in range(NC)], axis=0)
    return y.reshape(B, T, D).astype(x.dtype)
